# revision 23
# baseline (speedup 1.0000x reference)
"""Trainium2 Bass kernel for nn_AttentionDot (double batch-axis softmax attention).

Computation (B=4, N=M=4096, D=128, fp32):
    scores[b,n,m] = sum_d k[b,n,d] * q[b,m,d]
    w = softmax(softmax(scores, axis=0), axis=0)      # over batch axis (size 4)
    out[b,n,d]  = sum_m w[b,n,m] * v[b,m,d]

Sharding: N (rows of k / rows of scores) split across 8 NeuronCores; q, v
replicated. Each core computes its [B, 512, M] score slab, the axis-0
softmax (local - all 4 batch entries present), and its [B, 512, D] output
slab independently. No collectives.

v3 (TimelineSim 166us vs 362us baseline): the loop is ScalarE-bound (the
exp work: 2 passes over 2048 elements/chunk at 1 elem/cycle/lane) and
software-pipelined so ScalarE runs gap-free. Stages per iteration i:
    c  = i   : scores (PE, bf16 x bf16, 2 at a time into the two banks of
               one PSUM tile) + exp1 as two [128,1024] ACT ops
    c1 = i-1 : sum1 (PE identity-matmul accumulation) / rcp1 (DVE) /
               bf16 cvt (Pool) / normalize mul1 (DVE)
    c2 = i-2 : exp2, two in-place [128,1024] halves interleaved with the
               exp1 halves (they fill the score-matmul handoff gaps)
    c3 = i-3 : sum2 / rcp2 / cvt / mul2
    c4 = i-4 : out accumulation matmuls (PE -> 4 PSUM banks, whole loop)
PSUM = exactly 8 banks: scores 2 + ssum 1 + tsum 1 + out accumulators 4.

Data movement: q and v are cast fp32->bf16 by SWDGE DMAs whose access
patterns read contiguous 4-16KB blocks per partition, via the internal
permutations m = p*MCH + c (m is contracted away, so q/v only need to
agree) and n = p*NSUB + j (inverted by the output DMA pattern). q is then
transposed to [d, m] by X-bar transpose-DMAs. Tile serializes every
transpose-DMA against every other in-flight DMA (deadlock guard), so
casts and transposes are clustered into homogeneous blocks via explicit
add_dep_helper edges, and the later blocks are issued lazily inside the
loop so the Pool-engine converts don't queue behind descriptor
generation. k is loaded fp32 over HWDGE (parallel SP queue) and
PE-transposed in the preamble. The softmax chain runs in bf16 (~0.4% rel
err, gate is 2e-2).

Execution: each core runs the same single-device NEFF via its own PJRT
dispatch (async, all 8 in flight) - the multi-device shard_map executable
path wedges the axon terminal, so it is deliberately avoided.
"""

import numpy as np

import concourse.bass as bass
import concourse.tile as tile
from concourse import bacc, mybir, masks

B, N, M, D = 4, 4096, 4096, 128
NCORES = 8
NSH = N // NCORES            # 512 k-rows per core
NSUB = NSH // 128            # 4 partition-tiles of n
MCH = M // 128               # 32 m-chunks

F32 = mybir.dt.float32
BF16 = mybir.dt.bfloat16
EXP = mybir.ActivationFunctionType.Exp


def build_nc():
    nc = bacc.Bacc(
        "TRN2",
        target_bir_lowering=False,
        debug=False,
        enable_asserts=False,
        num_devices=NCORES,
    )
    kk = nc.dram_tensor("k", [B, NSH, D], F32, kind="ExternalInput").ap()
    qq = nc.dram_tensor("q", [B, M, D], F32, kind="ExternalInput").ap()
    vv = nc.dram_tensor("v", [B, M, D], F32, kind="ExternalInput").ap()
    out = nc.dram_tensor("out", [B, NSH, D], F32, kind="ExternalOutput").ap()

    from contextlib import ExitStack

    with tile.TileContext(nc) as tc, ExitStack() as ctx:
        const_pool = ctx.enter_context(tc.tile_pool(name="const", bufs=1))
        ident = const_pool.tile([128, 128], F32)
        masks.make_identity(nc, ident[:])
        identb = const_pool.tile([128, 128], BF16)
        nc.vector.tensor_copy(identb[:], ident[:])

        # m is fully contracted, so q and v share an internal m-permutation
        # m = p*MCH + c chosen to make every DMA read contiguous 4-16KB blocks
        # per partition (128 descriptors instead of 1024+). n is externally
        # visible, so its permutation n = p*NSUB + j is inverted by the output
        # DMA's access pattern.
        big = ctx.enter_context(tc.tile_pool(name="big", bufs=1))
        qnat = big.tile([128, B, MCH, 128], BF16, tag="qnat")  # [m_p, b, c, d]
        qT = big.tile([128, B, MCH, 128], BF16, tag="qT")      # [d, b, c, m_p]
        knat = big.tile([128, B, NSUB, 128], BF16, tag="knat")  # [n_p, b, j, d]
        kT = big.tile([128, B, NSUB, 128], BF16, tag="kT")     # [d, b, j, n_p]
        vS = big.tile([128, B, MCH, 128], BF16, tag="v")       # [m_p, b, c, d]
        outT = big.tile([128, B, NSH], F32, tag="outT")        # [d, b, n]

        # PSUM: exactly 8 banks.
        psS_pool = ctx.enter_context(tc.tile_pool(name="psS", bufs=1, space="PSUM"))
        psS = psS_pool.tile([128, 1024], F32, tag="s")         # 2 banks, recycled
        psW1 = ctx.enter_context(tc.tile_pool(name="psW1", bufs=1, space="PSUM"))
        psW2 = ctx.enter_context(tc.tile_pool(name="psW2", bufs=1, space="PSUM"))
        psO = ctx.enter_context(tc.tile_pool(name="psO", bufs=1, space="PSUM"))
        outps = [
            psO.tile([128, 512], F32, tag=f"o{b}", name=f"outps{b}") for b in range(B)
        ]

        # ---- input loads. All cast-DMAs (SWDGE fp32->bf16, contiguous
        # per-partition reads thanks to the m/n permutations) pipeline at
        # ~1us each; X-bar transpose-DMAs pipeline among themselves but Tile
        # serializes any transpose against any other in-flight DMA, so casts
        # and transposes are issued in large homogeneous blocks and only the
        # block boundaries pay the round-trip.
        QSP = 4
        qre = qq.rearrange("b (p c) d -> b p c d", c=MCH)
        vre = vv.rearrange("b (p c) d -> b p c d", c=MCH)
        g0 = slice(0, MCH // QSP)
        rest = slice(MCH // QSP, MCH)
        # cast block 1: k, q quarter 0   (unblocks the first chunks fast)
        for b in range(B):
            nc.gpsimd.dma_start(
                knat[:, b], kk[b].rearrange("(p j) d -> p j d", j=NSUB)
            )
        for b in range(B):
            nc.gpsimd.dma_start(qnat[:, b, g0], qre[b, :, g0])
        # transpose block 1: kT, qT quarter 0
        for b in range(B):
            nc.sync.dma_start_transpose(kT[:, b], knat[:, b])
        for b in range(B):
            nc.sync.dma_start_transpose(qT[:, b, g0], qnat[:, b, g0])
        # cast block 2: v quarter 0, q quarters 1-3
        for b in range(B):
            nc.gpsimd.dma_start(vS[:, b, g0], vre[b, :, g0])
        for b in range(B):
            nc.gpsimd.dma_start(qnat[:, b, rest], qre[b, :, rest])
        # transpose block 2: qT quarters 1-3
        for b in range(B):
            nc.sync.dma_start_transpose(qT[:, b, rest], qnat[:, b, rest])
        # cast block 3: v quarters 1-3
        for b in range(B):
            nc.gpsimd.dma_start(vS[:, b, rest], vre[b, :, rest])

        # ---- software-pipelined main loop over m-chunks ---------------------
        e_tiles = {}
        with tc.tile_pool(name="soft", bufs=6) as soft, tc.tile_pool(
            name="stat", bufs=2
        ) as stat:
            for i in range(MCH + 3):
                c = i          # stage 1: scores + exp1 + sum1/rcp1/mul1
                c2 = i - 2     # stage 2: exp2 (halves)
                c3 = i - 3     # tail: sum2/rcp2/mul2/out-matmuls

                if c < MCH:
                    e = soft.tile([128, B, 512], BF16, tag="e", name=f"e{c}")
                    e_tiles[c] = e
                    # scores b0, b1 -> the two banks of psS
                    nc.tensor.matmul(
                        psS[:, 0:512], qT[:, 0, c], kT[:, 0],
                        start=True, stop=True,
                    )
                    nc.tensor.matmul(
                        psS[:, 512:1024], qT[:, 1, c], kT[:, 1],
                        start=True, stop=True,
                    )
                    # exp1 of b01: one [128,1024] ACT op across both banks
                    nc.scalar.activation(
                        e[:, 0:2].rearrange("p b n -> p (b n)"), psS[:], EXP
                    )
                if 0 <= c2 < MCH:
                    eh = e_tiles[c2][:, 0:2].rearrange("p b n -> p (b n)")
                    nc.scalar.activation(eh, eh, EXP)
                if c < MCH:
                    e = e_tiles[c]
                    nc.tensor.matmul(
                        psS[:, 0:512], qT[:, 2, c], kT[:, 2],
                        start=True, stop=True,
                    )
                    nc.tensor.matmul(
                        psS[:, 512:1024], qT[:, 3, c], kT[:, 3],
                        start=True, stop=True,
                    )
                    nc.scalar.activation(
                        e[:, 2:4].rearrange("p b n -> p (b n)"), psS[:], EXP
                    )
                if 0 <= c2 < MCH:
                    eh = e_tiles[c2][:, 2:4].rearrange("p b n -> p (b n)")
                    nc.scalar.activation(eh, eh, EXP)

                if c3 >= 0:
                    e3 = e_tiles[c3]
                    tsum = psW2.tile([128, 512], F32, tag="t", name=f"ts{c3}")
                    for b in range(B):
                        nc.tensor.matmul(
                            tsum[:], identb[:], e3[:, b],
                            start=(b == 0), stop=(b == 3),
                        )
                    r2 = stat.tile([128, 512], F32, tag="r2", name=f"r2_{c3}")
                    nc.vector.reciprocal_approx_fast(r2[:], tsum[:])
                    r2b = stat.tile([128, 512], BF16, tag="r2b", name=f"r2b{c3}")
                    nc.vector.tensor_copy(r2b[:], r2[:])
                    nc.vector.tensor_mul(
                        e3[:], e3[:], r2b[:].unsqueeze(1).broadcast_to([128, B, 512])
                    )
                    for b in range(B):
                        nc.tensor.matmul(
                            outps[b][:], vS[:, b, c3], e3[:, b],
                            start=(c3 == 0), stop=(c3 == MCH - 1),
                        )

                if c < MCH:
                    e = e_tiles[c]
                    ssum = psW1.tile([128, 512], F32, tag="s", name=f"ss{c}")
                    for b in range(B):
                        nc.tensor.matmul(
                            ssum[:], identb[:], e[:, b],
                            start=(b == 0), stop=(b == 3),
                        )
                    r1 = stat.tile([128, 512], F32, tag="r1", name=f"r1_{c}")
                    nc.vector.reciprocal_approx_fast(r1[:], ssum[:])
                    r1b = stat.tile([128, 512], BF16, tag="r1b", name=f"r1b{c}")
                    nc.vector.tensor_copy(r1b[:], r1[:])
                    nc.vector.tensor_mul(
                        e[:], e[:], r1b[:].unsqueeze(1).broadcast_to([128, B, 512])
                    )

        # ---- epilogue: psO -> SBUF, transpose [d,n] -> [n,d], store ---------
        # b even/odd alternate between the two psS banks so the per-b chains
        # (DVE copy -> PE transposes -> DVE copy -> DMA) overlap.
        with tc.tile_pool(name="epi", bufs=4) as epi:
            for b in range(B):
                # ScalarE is drained by now; do the PSUM evacuations there so
                # they overlap the DVE onat copies
                nc.scalar.copy(outT[:, b], outps[b][:])
                half = (b % 2) * 512
                for j in range(NSUB):
                    nc.tensor.transpose(
                        psS[:, half + j * 128 : half + (j + 1) * 128],
                        outT[:, b, j * 128 : (j + 1) * 128],
                        ident[:],
                    )
                onat = epi.tile([128, NSUB, 128], F32, tag="onat", name=f"onat{b}")
                nc.vector.tensor_copy(
                    onat[:],
                    psS[:, half : half + 512].rearrange("p (j d) -> p j d", j=NSUB),
                )
                nc.sync.dma_start(
                    out[b].rearrange("(p j) d -> p j d", j=NSUB), onat[:]
                )

    nc.compile()
    return nc


# ---------------------------------------------------------------------------
# host-side execution

_NC_CACHE = None
LAST_RESULTS = None
LAST_EXEC_NS = None
LAST_PATH = None


def _with_timeout(fn, secs):
    """Run fn in a daemon thread with a deadline; raises TimeoutError.
    A hung remote fetch cannot be cancelled - the thread is leaked."""
    import threading

    box = {}

    def run():
        try:
            box["val"] = fn()
        except BaseException as e:  # noqa: BLE001
            box["err"] = e

    th = threading.Thread(target=run, daemon=True)
    th.start()
    th.join(secs)
    if "val" in box:
        return box["val"]
    if "err" in box:
        raise box["err"]
    raise TimeoutError(f"timed out after {secs}s")


def _run_spmd_native(nc, in_maps):
    """Native hardware path (real /dev/neuron*): the stock 8-core runner."""
    from concourse.bass_utils import run_bass_kernel_spmd

    res = run_bass_kernel_spmd(nc, in_maps, core_ids=list(range(NCORES)))
    global LAST_EXEC_NS
    if res.exec_time_ns is not None:
        LAST_EXEC_NS = res.exec_time_ns
    return res.results


def _run_per_device_axon(nc, in_maps):
    """Axon path: run the (collective-free) NEFF on each core as an
    independent single-device PJRT execution via the stock 1-core runner.
    The 8-device shard_map executable is avoided (it can wedge the axon
    terminal). Device 0 doubles as the compile probe: if it doesn't come
    back within its budget the whole path is abandoned."""
    import jax
    from concourse import bass2jax

    devs = jax.devices()
    results = []
    for c in range(NCORES):
        def call(c=c):
            with jax.default_device(devs[c]):
                return bass2jax.run_bass_via_pjrt(nc, [in_maps[c]], n_cores=1)

        # first call pays the NEFF compile; later calls reuse the cache
        results.append(_with_timeout(call, 1200 if c == 0 else 240)[0])
    return results


def _run_coresim(nc, in_maps):
    """Pure-simulation fallback: numerically correct, no hardware."""
    from concourse.bass_interp import CoreSim

    results = []
    for c in range(NCORES):
        sim = CoreSim(nc, trace=False, require_finite=False, require_nnan=False)
        for name, arr in in_maps[c].items():
            sim.tensor(name)[:] = arr
        sim.simulate(check_with_hw=False)
        results.append({"out": np.array(sim.tensor("out"))})
    return results


def kernel(k, q, v, _trace=False):
    global _NC_CACHE, LAST_RESULTS, LAST_PATH
    k = np.ascontiguousarray(np.asarray(k, dtype=np.float32))
    q = np.ascontiguousarray(np.asarray(q, dtype=np.float32))
    v = np.ascontiguousarray(np.asarray(v, dtype=np.float32))
    assert k.shape == (B, N, D) and q.shape == (B, M, D) and v.shape == (B, M, D)

    if _NC_CACHE is None:
        _NC_CACHE = build_nc()
    nc = _NC_CACHE

    in_maps = [
        {
            "k": np.ascontiguousarray(k[:, i * NSH : (i + 1) * NSH, :]),
            "q": q,
            "v": v,
        }
        for i in range(NCORES)
    ]

    from concourse._compat import axon_active

    attempts = []
    if axon_active():
        attempts.append(("axon-per-device", lambda: _run_per_device_axon(nc, in_maps), 2400))
    else:
        attempts.append(("native-spmd", lambda: _run_spmd_native(nc, in_maps), 2400))

    results = None
    for name, fn, budget in attempts:
        try:
            results = _with_timeout(fn, budget)
            LAST_PATH = name
            break
        except BaseException as e:  # noqa: BLE001
            import sys

            print(f"kernel: {name} failed ({e!r}); falling back", file=sys.stderr)
    if results is None:
        results = _run_coresim(nc, in_maps)
        LAST_PATH = "coresim"

    LAST_RESULTS = results
    return np.concatenate([r["out"] for r in results], axis=1)
